# revision 43
# baseline (speedup 1.0000x reference)
"""DisparityConv kernel for 8 Trainium2 NeuronCores.

Full inputs: x[8,32,256,512] f32, W[64,32,3,3] f32, bias[64] f32.
Data-parallel over batch: core i computes x[i] -> out[i] [64,256,512].

Per-core pipeline:
  xe (bf16, width-extended by S for the circular roll) prepared host-side.
  Partition layout (j,c): 4 stagger-replicas (XR baked shift j+1) x 32 channels.
  Per shift-group g (delta=4g): |XR[.., w+4g] - X4[.., w]| gives abs-diffs for
  shifts s=4g+j+1 on partition group j. Abs-diff runs on a schedule across:
    - fused custom DVE op ABS_DIFF (1x rate, one op)
    - DVE subtract (2x bf16) + ScalarE activation(Abs)
    - GPSIMD subtract + ScalarE activation(Abs)
  Channel mean + shift departition: mask matmul on PE -> psum[(jrep,s), w],
  jrep replicas 0..2 become the kh=0..2 blocks of the conv's K=96 operand.
  ScalarE casts psum[0:96] -> Dstage bf16; 3 SBUF-SBUF DMAs scatter the three
  kh blocks onto a DIAGONAL ring D3h (block kh of diff row r lands at slot
  (r+1-kh)%R), so conv for output row rr reads the single uniform slot rr%R
  and gets rows rr-1, rr, rr+1 stacked on partitions: 3 dense K=96 matmuls
  (one per kw, rhs w-offset) accumulating in PSUM, col-split by row parity.
  Bias added by ScalarE on PSUM eviction; f32 DMA out.
"""
import sys

sys.path.insert(0, "/opt/trn_rl_repo")

import numpy as np
import ml_dtypes

import concourse.bass as bass  # noqa: F401
import concourse.tile as tile
from concourse import bacc, mybir
from concourse import bass_utils
from concourse import dve_ops
from concourse.dve_ops import DveOp
from concourse.dve_spec import Spec, Src0, Src1, Bin, maxx, lower, _has_src1
from concourse.dve_uop import (
    DveOpSpec, UopConfig, AluOp as UAluOp, AluInp, DelayInp, InpSel,
    OutSel, OutPath, Trigger, ENABLE,
)

F32 = mybir.dt.float32
BF16 = mybir.dt.bfloat16
Alu = mybir.AluOpType
Act = mybir.ActivationFunctionType

B = 8
C = 32
S = 32
O = 64
SG = S // 4
FULL_H, FULL_W = 256, 512
N_CORES = 8

# Per-(block,group) engine schedule, cycled mod len:
# 'f' = fused DVE ABS_DIFF (2x perf mode: hand-written 2X_1PORT uop
#       program using the native ABSOLUTE_DIFF ALU op -- one DVE pass
#       at 2 bf16/lane/cycle does sub+abs, same cost as a bare sub),
# 'p' = PE identity-matmul sub (psum = I.T@xr - I.T@x4 per row) + ACT
#       abs on eviction: relieves DVE entirely AND keeps the PE busy
#       enough between mask matmuls that HAM stays at full clock,
# 'v' = DVE sub + ACT abs, 'i' = DVE sub + DVE int16 abs (4x mode),
# 'g' = GPSIMD sub + ACT abs (AVOID: measured 60% slowdown of all
#       concurrent DVE ops from the shared SBUF port pair)
ABS_SCHEDULE = "ffffffffffffffff"


def _uops_2x_absdiff():
    """2X_1PORT uop program for |src0 - src1|, mirroring the stock
    tensor_tensor 2x program (gen3 firmware table slot 9) with the ALU op
    pinned to ABSOLUTE_DIFF: the engine reads one 32-bit word per port per
    cycle (two packed bf16), block0 computes elem0, block1 computes elem1
    (HI halves routed via delay chains 1/2), block2 swaps elem0 onto the
    ALU chain and elem1 onto delay chain 0, and the write stage emits
    WR0_LO=ALU_OUT (elem0), WR0_HI=DELAY_0 (elem1)."""
    u = UopConfig()
    for lane, src in enumerate((InpSel.SRC_0, InpSel.SRC_1,
                                InpSel.SRC_0_HI, InpSel.SRC_1_HI)):
        u.inp[lane] = src
        u.inp_enable[lane] = ENABLE
    u.require_inp0 = ENABLE
    u.require_inp1 = ENABLE
    u.trigger = (Trigger.SRC_TENSOR_DONE, Trigger.NONE, Trigger.NONE)
    u.out[OutPath.WR0_LO] = OutSel.ALU_OUT
    u.out_enable[OutPath.WR0_LO] = ENABLE
    u.out[OutPath.WR0_HI] = OutSel.DELAY_0
    u.out_enable[OutPath.WR0_HI] = ENABLE
    dp = u.datapath_config
    dp[0].enable_alu(UAluOp.ABSOLUTE_DIFF, AluInp.PREV_ALU_OUT,
                     AluInp.PREV_DELAY_0)
    dp[0].pass_through_delay(1, 2)
    dp[1].enable_alu(UAluOp.ABSOLUTE_DIFF, AluInp.PREV_DELAY_1,
                     AluInp.PREV_DELAY_2)
    dp[1].enable_delay_from_src(DelayInp.PREV_ALU_OUT, 0)
    dp[2].enable_alu(UAluOp.BYPASS, AluInp.PREV_DELAY_0, AluInp.PREV_DELAY_0)
    dp[2].enable_delay_from_src(DelayInp.PREV_ALU_OUT, 0)
    for b in range(3, 8):
        dp[b].pass_through_alu()
        dp[b].pass_through_delay(0)
    return [u]


def _register_abs_diff():
    if "ABS_DIFF_ANT" in dve_ops._SUB_OPCODE_FOR_NAME:
        return dve_ops._ABS_DIFF_ANT_OP
    # reference/interp semantics (also the 1x fallback program): |a - b|
    spec = Spec(
        body=Bin(UAluOp.ABSOLUTE_DIFF, Src0, Src1),
        reference=lambda in0, in1, s0, s1, imm2: np.abs(
            in0.astype(np.float32)
            - in1.astype(np.float32).reshape(in0.shape)
        ),
    )
    row = dve_ops._CUSTOM_DVE_ROW_BASE + len(dve_ops.OPS)
    assert row < 0x20
    op = DveOp("ABS_DIFF_ANT", spec, subdim=False, uops_sha={})
    dve_ops._SUB_OPCODE_FOR_NAME["ABS_DIFF_ANT"] = row
    dve_ops.OPS.append(op)
    dve_ops.CUSTOM_DVE_SPECS["ABS_DIFF_ANT"] = spec
    compiled = DveOpSpec(
        name="ABS_DIFF_ANT", opcode=row, uops=lower(spec, ver="v3"),
        uops_2x=_uops_2x_absdiff(), perf_max=1, rd1_en=_has_src1(spec),
    )
    op.uops_sha["v3"] = compiled.sha("v3")
    # Seed the compile cache so DveOp.compile / dve_table_for_ops pick up
    # the hand-authored 2x variant (lower() alone only emits the 1x program).
    dve_ops._COMPILE_CACHE[("ABS_DIFF_ANT", "v3")] = compiled
    dve_ops._ABS_DIFF_ANT_OP = op
    # byte-36[7:6] perf_max must be nonzero on the *instruction* for the
    # engine to consider perf modes; _custom_dve hardcodes 0, so intercept
    # the ISA constructor for this op's emissions.
    if not getattr(bass.bass_isa, "_ant_absdiff_perfmax_patch", False):
        orig_ctor = bass.bass_isa.InstCustomDveAnt

        def _ctor(**kw):
            if kw.get("op_name") == "ABS_DIFF_ANT":
                kw["perf_max"] = 1
            return orig_ctor(**kw)

        bass.bass_isa.InstCustomDveAnt = _ctor
        bass.bass_isa._ant_absdiff_perfmax_patch = True
    return op


def _build_nc(H=FULL_H, W=FULL_W, hb=8, R=12, num_devices=N_CORES):
    absd = _register_abs_diff()
    WE = W + S
    nc = bacc.Bacc("TRN2", target_bir_lowering=False, debug=False,
                   num_devices=num_devices)

    # xe carries one junk pad row so the flat-contiguous shifted loads can
    # overrun the last block by up to S elements
    xe = nc.dram_tensor("xe", [C, H + 1, WE], BF16, kind="ExternalInput").ap()
    xe_flat = xe.rearrange("c h w -> c (h w)")
    ident = nc.dram_tensor("ident", [2, 128, 128], BF16,
                           kind="ExternalInput").ap()
    masks = nc.dram_tensor("masks", [SG, 128, 128], BF16, kind="ExternalInput").ap()
    convw = nc.dram_tensor("convw", [3, 96, O], BF16, kind="ExternalInput").ap()
    bias2 = nc.dram_tensor("bias2", [128, 1], F32, kind="ExternalInput").ap()
    out = nc.dram_tensor("out", [O, H, W], F32, kind="ExternalOutput").ap()
    out_hv = out.rearrange("o h w -> h o w")

    assert H % hb == 0 and H % 2 == 0
    nblk = H // hb

    with tile.TileContext(nc) as tc:
        with (
            tc.tile_pool(name="const", bufs=1) as constp,
            tc.tile_pool(name="stage", bufs=2) as stagep,
            tc.tile_pool(name="ap", bufs=2 * SG) as apool,
            tc.tile_pool(name="dst", bufs=4) as dstp,
            tc.tile_pool(name="d4p", bufs=1) as d4p,
            tc.tile_pool(name="outp", bufs=4) as outp,
            tc.tile_pool(name="psd", bufs=2, space="PSUM") as psdp,
            tc.tile_pool(name="pso", bufs=4, space="PSUM") as psop,
        ):
            maskT = constp.tile([128, SG * 128], BF16)
            for g in range(SG):
                nc.sync.dma_start(maskT[:, g * 128:(g + 1) * 128], masks[g])
            WT = constp.tile([96, 3 * O], BF16)
            for kw in range(3):
                nc.sync.dma_start(WT[:, kw * O:(kw + 1) * O], convw[kw])
            biasT = constp.tile([128, 1], F32)
            nc.sync.dma_start(biasT[:], bias2[:])
            idT = constp.tile([128, 2 * 128], BF16)
            for ii in range(2):
                nc.sync.dma_start(idT[:, ii * 128:(ii + 1) * 128], ident[ii])

            d3 = d4p.tile([96, R * (W + 2)], BF16)
            d3v = d3.rearrange("p (r w) -> p r w", w=W + 2)
            # slot-pair view: slot = 2*a + b
            d3p = d3.rearrange("p (a b w) -> p a b w", b=2, w=W + 2)
            for sl in range(R):
                nc.vector.memset(d3v[:, sl, :], 0)

            def conv_pairs(p0s):
                # interleave the matmul streams of up to 2 row-pairs so one
                # accumulation chain's drain hides under the other's fill
                psos = {p0: psop.tile([128, W], F32, tag="pso", name=f"pso{p0}")
                        for p0 in p0s}
                for half in (0, 1):
                    for kw in (0, 1, 2):
                        for p0 in p0s:
                            rr = p0 + half
                            nc.tensor.matmul(
                                psos[p0][64 * half:64 * half + 64, :],
                                WT[:, kw * O:kw * O + O],
                                d3v[:, rr % R, kw:kw + W],
                                start=(kw == 0), stop=(kw == 2),
                                tile_position=(0, 64 * half),
                            )
                for p0 in p0s:
                    ot = outp.tile([128, W], F32)
                    # bias-add eviction on ACT (Identity spline + per-partition
                    # bias AP): DVE is the kernel bottleneck, ScalarE has slack
                    nc.scalar.activation(ot[:], psos[p0][:], Act.Identity,
                                         bias=biasT[:])
                    # result stores ride the (lightly loaded) SWDGE queue so
                    # they cannot head-of-line-block the input loads on sync
                    nc.gpsimd.dma_start(out_hv[p0:p0 + 2], ot[:])

            nsched = len(ABS_SCHEDULE)

            def load_block(blk):
                h0 = blk * hb
                # full-row (stride WE) staging tiles loaded as flat
                # contiguous slices of xe -- 32 large descriptors per DMA
                # instead of 256 row fragments; the j+1 stagger shift is
                # baked into the flat source offset
                x4 = stagep.tile([128, hb * WE], BF16, tag="x4",
                                 name=f"x4_{blk}")
                xr = stagep.tile([128, hb * WE], BF16, tag="xr",
                                 name=f"xr_{blk}")
                n = hb * WE
                nc.sync.dma_start(x4[0:32, :],
                                  xe_flat[:, WE * h0:WE * h0 + n])
                nc.sync.dma_start(xr[0:32, :],
                                  xe_flat[:, WE * h0 + 1:WE * h0 + 1 + n])
                # stagger replicas as SBUF-SBUF copies of the j=0 rows at
                # elem offsets 0..3 (replica rows only ever read up to elem
                # (hb-1)*WE + 539, so width n-4 stays in-bounds)
                for j in range(1, 4):
                    nc.sync.dma_start(x4[32 * j:32 * j + 32, :],
                                      x4[0:32, :])
                    nc.sync.dma_start(xr[32 * j:32 * j + 32, 0:n - 4],
                                      xr[0:32, j:j + n - 4])
                return x4, xr

            def produce_block(blk, staged):
                x4, xr = staged
                x4v = x4.rearrange("p (h w) -> p h w", w=WE)
                xrv = xr.rearrange("p (h w) -> p h w", w=WE)
                x4w = x4v[:, :, 0:W]
                avs = []
                for g in range(SG):
                    mode = ABS_SCHEDULE[(blk * SG + g) % nsched]
                    a = apool.tile([128, hb * W], BF16, tag="a",
                                   name=f"a_{blk}_{g}")
                    av = a.rearrange("p (h w) -> p h w", w=W)
                    in0 = xrv[:, :, 4 * g:4 * g + W]
                    if mode == "f":
                        nc.vector._custom_dve(absd, out=av, in0=in0, in1=x4w)
                    elif mode == "i":
                        nc.vector.tensor_sub(av, in0, x4w)
                        ai = a[:].bitcast(mybir.dt.int16)
                        nc.vector.tensor_single_scalar(
                            ai, ai, 0x7FFF, Alu.bitwise_and)
                    elif mode == "v":
                        nc.vector.tensor_sub(av, in0, x4w)
                        nc.scalar.activation(a, a, Act.Abs)
                    else:
                        nc.gpsimd.tensor_sub(av, in0, x4w)
                        nc.scalar.activation(a, a, Act.Abs)
                    avs.append(av)
                return avs

            staged = {b: load_block(b) for b in range(min(2, nblk))}
            next_avs = produce_block(0, staged.pop(0))
            for blk in range(nblk):
                h0 = blk * hb
                avs = next_avs
                if blk + 2 < nblk:
                    staged[blk + 2] = load_block(blk + 2)
                if blk + 1 < nblk:
                    next_avs = produce_block(blk + 1, staged.pop(blk + 1))
                for rl in range(0, hb, 4):
                    # four rows' mask-matmul chains interleaved (independent
                    # accumulators) so drains overlap fills; paired rows
                    # share a [128, 2W] tile so eviction is 2 wide ACT
                    # copies instead of 4 narrow ones
                    psd2 = [psdp.tile([128, 2 * W], F32, tag="psd",
                                      name=f"psd{blk}_{rl}_{h2}")
                            for h2 in range(2)]
                    for g in range(SG):
                        for q in range(4):
                            nc.tensor.matmul(
                                psd2[q // 2][:, (q % 2) * W:(q % 2) * W + W],
                                maskT[:, g * 128:(g + 1) * 128],
                                avs[g][:, rl + q, :],
                                start=(g == 0), stop=(g == SG - 1),
                            )
                    ds4 = dstp.tile([96, 4 * W], BF16, tag="ds4",
                                    name=f"ds4_{blk}_{rl}")
                    ds4v = ds4.rearrange("p (a w) -> p a w", w=W)
                    for h2 in range(2):
                        nc.scalar.copy(ds4[:, h2 * 2 * W:(h2 + 1) * 2 * W],
                                       psd2[h2][0:96, :])
                    r = h0 + rl + 3
                    # batched diagonal-ring scatter: rows r-3..r, per kh
                    # block kh of diff row q lands at slot (q+1-kh)%R
                    r0 = r - 3
                    for kh in range(3):
                        s0 = (r0 + 1 - kh) % R
                        n1 = min(4, R - s0)
                        for (a0, sl0, cnt) in (((0, s0, n1),) if n1 == 4 else
                                               ((0, s0, n1), (n1, 0, 4 - n1))):
                            nc.gpsimd.dma_start(
                                d3v[32 * kh:32 * kh + 32,
                                    sl0:sl0 + cnt, 1:W + 1],
                                ds4v[32 * kh:32 * kh + 32,
                                     a0:a0 + cnt, :])
                    due = [p0 for p0 in (r - 9, r - 7) if 0 <= p0 <= H - 4]
                    if due:
                        conv_pairs(due)
            # zero the kh=2 slot that would hold (nonexistent) diff row H
            nc.vector.memset(d3v[64:96, (H - 1) % R, 1:W + 1], 0)
            conv_pairs([H - 6, H - 4])
            conv_pairs([H - 2])

    nc.compile()
    return nc


_NC_CACHE = {}


def _get_nc():
    if "nc" not in _NC_CACHE:
        _NC_CACHE["nc"] = _build_nc()
    return _NC_CACHE["nc"]


def host_prep_shared(Wc, bias):
    bf16 = ml_dtypes.bfloat16
    ident = np.stack([np.eye(128, dtype=np.float32),
                      -np.eye(128, dtype=np.float32)]).astype(bf16)
    masks = np.zeros((SG, 128, 128), np.float32)
    for g in range(SG):
        for j in range(4):
            for jr in range(4):
                masks[g, 32 * j:32 * j + 32, 32 * jr + 4 * g + j] = 1.0 / C
    masks = masks.astype(bf16)
    # convw[kw, 32*kh + s, o] = Wc[o, s, kh, kw]
    convw = np.ascontiguousarray(
        Wc.transpose(3, 2, 1, 0).reshape(3, 96, O)).astype(bf16)
    bias2 = np.concatenate([bias, bias]).reshape(128, 1).astype(np.float32)
    return masks, convw, bias2, ident


def kernel(x, W, bias, _trace=False, _tmpdir=None):
    """x:[8,32,256,512] f32, W:[64,32,3,3] f32, bias:[64] f32 -> [8,64,256,512]."""
    nc = _get_nc()
    bf16 = ml_dtypes.bfloat16
    masks, convw, bias2, ident = host_prep_shared(np.asarray(W, np.float32),
                                           np.asarray(bias, np.float32))
    x = np.asarray(x, np.float32)
    xe_all = np.concatenate([x, x[:, :, :, :S]], axis=3).astype(bf16)
    # one junk pad row per channel (flat shifted loads overrun the last block)
    xe_all = np.concatenate(
        [xe_all, np.zeros_like(xe_all[:, :, :1, :])], axis=2)
    in_maps = [
        {"xe": xe_all[i], "masks": masks, "convw": convw, "bias2": bias2,
         "ident": ident}
        for i in range(N_CORES)
    ]
    kw = {}
    if _trace:
        kw = dict(trace=True, tmpdir=_tmpdir)
    res = bass_utils.run_bass_kernel_spmd(
        nc, in_maps, core_ids=list(range(N_CORES)), **kw)
    out = np.stack([res.results[i]["out"] for i in range(N_CORES)], axis=0)
    if _trace:
        kernel.last_exec_time_ns = res.exec_time_ns
        kernel.last_results = res
    return out

